# revision 15
# baseline (speedup 1.0000x reference)
"""Fused transformer block (nn_Block_2388001816768) on 8 Trainium2 NeuronCores.

Sharding: (batch, sequence-half) -> one core. Core c handles batch c//2,
query rows [o*1024:(o+1)*1024] where o = c%2. No collectives: each core
recomputes LN1 + K/V projections for the full sequence of its batch.

Per-core local sequence order is [own half | other half]; causality then
becomes: local triangle over half-1 (identical structure on every core,
handled at tile granularity + a tril constant on diagonal tiles), and
half-2 all-or-nothing (handled by a per-core additive bias in the exp).

All large matmuls run in float32r (TF32-like, full PE rate), fp32 accum.
LN scale/shift (g, b) are folded into the projection weights host-side;
the 1/sqrt(HD) score scale is folded into Wq/bq.
"""

import os

import numpy as np

import concourse.bacc as bacc
import concourse.bass as bass  # noqa: F401
import concourse.mybir as mybir
import concourse.tile as tile
from concourse.bass_utils import run_bass_kernel_spmd
from concourse.masks import make_identity

B, T, D, H = 4, 2048, 1024, 16
HD = D // H  # 64
FF = 4 * D  # 4096
TQ = T // 2  # rows per core = 1024
P = 128
NEG = -60000.0  # additive mask: exp(x + NEG) == 0 in fp32

f32 = mybir.dt.float32
f32r = mybir.dt.float32r
AF = mybir.ActivationFunctionType
ALU = mybir.AluOpType

_CACHE = {}


def _build_program():
    nc = bacc.Bacc(None, target_bir_lowering=False)

    xl_d = nc.dram_tensor("xl", (T, D), f32, kind="ExternalInput")
    wq_d = nc.dram_tensor("wq", (D, D), f32r, kind="ExternalInput")
    wk_d = nc.dram_tensor("wk", (D, D), f32r, kind="ExternalInput")
    wv_d = nc.dram_tensor("wv", (D, D), f32r, kind="ExternalInput")
    wo_d = nc.dram_tensor("wo", (D, D), f32r, kind="ExternalInput")
    w1_d = nc.dram_tensor("w1", (D, FF), f32r, kind="ExternalInput")
    w2_d = nc.dram_tensor("w2", (FF, D), f32r, kind="ExternalInput")
    qkvb_d = nc.dram_tensor("qkvb", (HD, 3 * H), f32, kind="ExternalInput")
    bo_d = nc.dram_tensor("bo_", (1, D), f32, kind="ExternalInput")
    b1f_d = nc.dram_tensor("b1f", (P, FF // P), f32, kind="ExternalInput")
    b2_d = nc.dram_tensor("b2_", (1, D), f32, kind="ExternalInput")
    h2b_d = nc.dram_tensor("h2b", (1, 1), f32, kind="ExternalInput")
    out_d = nc.dram_tensor("out", (TQ, D), f32, kind="ExternalOutput")

    DT = D // P  # 8 d-tiles
    NT = T // P  # 16 t-tiles
    NQ = TQ // P  # 8 q-tiles
    FT = FF // P  # 32 ff-tiles

    KDBG = os.environ.get("KDBG", "0") == "1"
    if KDBG:
        dbg_hT_d = nc.dram_tensor("dbg_hT", (P, DT, T), f32, kind="ExternalOutput")
        dbg_qT_d = nc.dram_tensor("dbg_qT", (HD, TQ), f32, kind="ExternalOutput")
        dbg_kT_d = nc.dram_tensor("dbg_kT", (HD, T), f32, kind="ExternalOutput")
        dbg_va_d = nc.dram_tensor("dbg_va", (P, NT, HD + 1), f32, kind="ExternalOutput")
        dbg_pt_d = nc.dram_tensor("dbg_pt", (2, P, 512), f32, kind="ExternalOutput")
        dbg_cx_d = nc.dram_tensor("dbg_cx", (HD + 1, 512), f32, kind="ExternalOutput")
        dbg_bc_d = nc.dram_tensor("dbg_bc", (HD, TQ), f32, kind="ExternalOutput")
        dbg_ct_d = nc.dram_tensor("dbg_ct", (P, DT, TQ), f32, kind="ExternalOutput")
        dbg_x2_d = nc.dram_tensor("dbg_x2", (NQ, P, D), f32, kind="ExternalOutput")

    with tile.TileContext(nc) as tc:
        with (
            tc.tile_pool(name="const", bufs=1) as const,
            tc.tile_pool(name="dramp", bufs=1, space="DRAM") as dramp,
        ):
            ident_f = const.tile([P, P], f32)
            make_identity(nc, ident_f)
            ident = const.tile([P, P], f32r)
            nc.vector.tensor_copy(ident, ident_f)
            # S^T-space causal keep mask: keep where kv(part) <= q(free)
            tril_f = const.tile([P, P], f32)
            nc.gpsimd.memset(tril_f, 1.0)
            nc.gpsimd.affine_select(
                out=tril_f, in_=tril_f, compare_op=ALU.is_ge, fill=0.0,
                base=0, pattern=[[1, P]], channel_multiplier=-1,
            )
            tril = const.tile([P, P], f32r)
            nc.vector.tensor_copy(tril, tril_f)
            ones16 = const.tile([P, NT], f32)
            nc.vector.memset(ones16, 1.0)
            qkvb = const.tile([HD, 3 * H], f32)
            nc.sync.dma_start(qkvb, qkvb_d[:, :])
            eps = const.tile([P, 1], f32)
            nc.vector.memset(eps, 1e-5)
            h2b = const.tile([P, 1], f32)
            nc.sync.dma_start(h2b, h2b_d.ap().to_broadcast([P, 1]))

            x2d = dramp.tile([NQ, P, D], f32)  # x2 spill (post-attn residual)

            with tc.tile_pool(name="ctxp", bufs=1) as ctxp:
                ctxT = ctxp.tile([P, DT, TQ], f32r)  # ctx^T head-pair-stacked

                with tc.tile_pool(name="hTp", bufs=1) as hTp:
                    hT = hTp.tile([P, DT, T], f32r)  # h^T [d%P, d//P, t]

                    # ---------- Phase 1: LN1 + transpose ----------
                    with (
                        tc.tile_pool(name="ln1", bufs=3) as ln1,
                        tc.tile_pool(name="ps1", bufs=2, space="PSUM") as ps1,
                    ):
                        for tt in range(NT):
                            x_t = ln1.tile([P, D], f32, tag="x_t")
                            nc.sync.dma_start(x_t, xl_d[tt * P:(tt + 1) * P, :])
                            st = ln1.tile([P, 2, 6], f32, tag="st")
                            nc.vector.bn_stats(st[:, 0, :], x_t[:, 0:512])
                            nc.vector.bn_stats(st[:, 1, :], x_t[:, 512:1024])
                            mv = ln1.tile([P, 2], f32, tag="mv")
                            nc.vector.bn_aggr(mv, st)
                            rstd = ln1.tile([P, 1], f32, tag="rstd")
                            nc.scalar.activation(rstd, mv[:, 1:2], AF.Sqrt, bias=eps)
                            nc.vector.reciprocal(rstd, rstd)
                            h_t = ln1.tile([P, D], f32r, tag="h_t")
                            nc.vector.tensor_scalar(
                                out=h_t, in0=x_t, scalar1=mv[:, 0:1],
                                scalar2=rstd, op0=ALU.subtract, op1=ALU.mult,
                            )
                            for dt in range(DT):
                                tp = ps1.tile([P, P], f32r, tag="tp")
                                nc.tensor.transpose(
                                    tp, h_t[:, dt * P:(dt + 1) * P], ident)
                                nc.scalar.copy(
                                    hT[:, dt, tt * P:(tt + 1) * P], tp)

                    if KDBG:
                        nc.sync.dma_start(dbg_hT_d.ap(), hT[:, :, :].bitcast(f32))

                    # ---------- Phase 2: per-head QKV + attention ----------
                    with (
                        tc.tile_pool(name="whead", bufs=2) as whead,
                        tc.tile_pool(name="head", bufs=2) as head,
                        tc.tile_pool(name="pt", bufs=4) as ptp,
                        tc.tile_pool(name="psqkv", bufs=2, space="PSUM") as psqkv,
                        tc.tile_pool(name="pssc", bufs=3, space="PSUM") as pssc,
                        tc.tile_pool(name="psctx", bufs=2, space="PSUM") as psctx,
                    ):
                        for h in range(H):
                            wq_h = whead.tile([P, DT, HD], f32r, tag="wq_h")
                            nc.sync.dma_start(
                                wq_h, wq_d[:, h * HD:(h + 1) * HD].rearrange(
                                    "(dt p) m -> p dt m", p=P))
                            wk_h = whead.tile([P, DT, HD], f32r, tag="wk_h")
                            nc.sync.dma_start(
                                wk_h, wk_d[:, h * HD:(h + 1) * HD].rearrange(
                                    "(dt p) m -> p dt m", p=P))
                            wv_h = whead.tile([P, DT, HD], f32r, tag="wv_h")
                            nc.sync.dma_start(
                                wv_h, wv_d[:, h * HD:(h + 1) * HD].rearrange(
                                    "(dt p) m -> p dt m", p=P))

                            qT = head.tile([HD, TQ], f32r, tag="qT")
                            kT = head.tile([HD, T], f32r, tag="kT")
                            vT = head.tile([HD, T], f32r, tag="vT")
                            for (w_h, dst, nchunk, bcol) in (
                                (wq_h, qT, TQ // 512, 3 * h + 0),
                                (wk_h, kT, T // 512, 3 * h + 1),
                                (wv_h, vT, T // 512, 3 * h + 2),
                            ):
                                for c in range(nchunk):
                                    pp = psqkv.tile([HD, 512], f32, tag="pp")
                                    for dt in range(DT):
                                        nc.tensor.matmul(
                                            pp, w_h[:, dt, :],
                                            hT[:, dt, c * 512:(c + 1) * 512],
                                            start=(dt == 0), stop=(dt == DT - 1))
                                    nc.scalar.activation(
                                        dst[:, c * 512:(c + 1) * 512], pp,
                                        AF.Identity, bias=qkvb[:, bcol:bcol + 1])

                            vaug = head.tile([P, NT, HD + 1], f32r, tag="vaug")
                            nc.vector.tensor_copy(
                                vaug[:, :, HD:HD + 1], ones16.unsqueeze(2))
                            for kt in range(NT):
                                vp = pssc.tile([P, 512], f32r, tag="sps")
                                nc.tensor.transpose(
                                    vp[:, 0:HD], vT[:, kt * P:(kt + 1) * P],
                                    ident[0:HD, 0:HD])
                                nc.vector.tensor_copy(vaug[:, kt, 0:HD], vp[:, 0:HD])

                            bc = head.tile([HD, TQ], f32, tag="bc")
                            rr = head.tile([HD + 1, 512], f32, tag="rr")
                            rr0 = head.tile([1, 512], f32, tag="rr0")
                            if KDBG and h == 0:
                                nc.sync.dma_start(dbg_qT_d.ap(), qT[:, :].bitcast(f32))
                                nc.sync.dma_start(dbg_kT_d.ap(), kT[:, :].bitcast(f32))
                                nc.sync.dma_start(dbg_va_d.ap(), vaug[:, :, :].bitcast(f32))
                            for qc in range(2):
                                qsl = slice(qc * 512, (qc + 1) * 512)
                                kv_tiles = (list(range(0, (qc + 1) * 4))
                                            + list(range(8, 16)))
                                ctx = psctx.tile([HD + 1, 512], f32, tag="ctx")
                                for n, i in enumerate(kv_tiles):
                                    sps = pssc.tile([P, 512], f32, tag="sps")
                                    nc.tensor.matmul(
                                        sps, kT[:, i * P:(i + 1) * P], qT[:, qsl],
                                        start=True, stop=True)
                                    pt = ptp.tile([P, 512], f32r, tag="pt")
                                    if i >= 8:  # other half: all-or-nothing
                                        nc.scalar.activation(
                                            pt, sps, AF.Exp, bias=h2b)
                                    else:
                                        jd = i - qc * 4  # diag column subtile
                                        if jd < 0:  # fully visible
                                            nc.scalar.activation(pt, sps, AF.Exp)
                                        else:
                                            if jd > 0:
                                                nc.vector.memset(
                                                    pt[:, 0:jd * P].bitcast(f32),
                                                    0.0)
                                            dsl = slice(jd * P, (jd + 1) * P)
                                            nc.scalar.activation(
                                                pt[:, dsl], sps[:, dsl], AF.Exp)
                                            nc.vector.tensor_mul(
                                                pt[:, dsl], pt[:, dsl], tril)
                                            if jd < 3:
                                                rsl = slice((jd + 1) * P, 512)
                                                nc.scalar.activation(
                                                    pt[:, rsl], sps[:, rsl],
                                                    AF.Exp)
                                    if KDBG and h == 0 and qc == 0 and n < 2:
                                        nc.sync.dma_start(dbg_pt_d[n], pt.bitcast(f32))
                                    nc.tensor.matmul(
                                        ctx, vaug[:, i, :], pt,
                                        start=(n == 0),
                                        stop=(n == len(kv_tiles) - 1))
                                if KDBG and h == 0 and qc == 0:
                                    cx_t = ptp.tile([HD + 1, 512], f32, tag="cxd")
                                    nc.vector.tensor_copy(cx_t, ctx)
                                    nc.sync.dma_start(dbg_cx_d.ap(), cx_t)
                                # softmax normalization: 1/rowsum, bcast
                                nc.vector.reciprocal(
                                    rr[HD:HD + 1, :], ctx[HD:HD + 1, :])
                                nc.sync.dma_start(rr0, rr[HD:HD + 1, :])
                                nc.gpsimd.partition_broadcast(
                                    bc[:, qsl], rr0)
                                pc = h // 2
                                if h % 2 == 0:
                                    nc.vector.tensor_mul(
                                        ctxT[0:HD, pc, qsl], ctx[0:HD, :],
                                        bc[:, qsl])
                                else:
                                    tmp = ptp.tile([HD, 512], f32r, tag="tmp")
                                    nc.vector.tensor_mul(
                                        tmp, ctx[0:HD, :], bc[:, qsl])
                                    nc.sync.dma_start(ctxT[HD:P, pc, qsl], tmp)

                if KDBG:
                    nc.sync.dma_start(dbg_ct_d.ap(), ctxT[:, :, :].bitcast(f32))

                # ---------- Phase 3: Wo proj + residual -> x2 (DRAM) ----------
                with (
                    tc.tile_pool(name="wop", bufs=1) as wop,
                    tc.tile_pool(name="ph3", bufs=3) as ph3,
                    tc.tile_pool(name="ps3", bufs=2, space="PSUM") as ps3,
                ):
                    wo_sb = wop.tile([P, DT, D], f32r)
                    nc.sync.dma_start(
                        wo_sb, wo_d.ap().rearrange("(pc p) n -> p pc n", p=P))
                    bo_bc = wop.tile([P, D], f32)
                    nc.sync.dma_start(bo_bc, bo_d.ap().to_broadcast([P, D]))
                    for qt in range(NQ):
                        xo_t = ph3.tile([P, D], f32, tag="xo_t")
                        nc.sync.dma_start(xo_t, xl_d[qt * P:(qt + 1) * P, :])
                        x2_t = ph3.tile([P, D], f32, tag="x2_t")
                        for dc in range(2):
                            dsl = slice(dc * 512, (dc + 1) * 512)
                            acc = ps3.tile([P, 512], f32, tag="acc")
                            for pc in range(DT):
                                nc.tensor.matmul(
                                    acc, ctxT[:, pc, qt * P:(qt + 1) * P],
                                    wo_sb[:, pc, dsl],
                                    start=(pc == 0), stop=(pc == DT - 1))
                            nc.vector.tensor_add(x2_t[:, dsl], acc, xo_t[:, dsl])
                            nc.vector.tensor_add(
                                x2_t[:, dsl], x2_t[:, dsl], bo_bc[:, dsl])
                        nc.sync.dma_start(x2d[qt], x2_t)
                        if KDBG:
                            nc.sync.dma_start(dbg_x2_d[qt], x2_t)

            # ---------- Phase 4: LN2 + transpose ----------
            with tc.tile_pool(name="h2Tp", bufs=1) as h2Tp:
                h2T = h2Tp.tile([P, DT, TQ], f32r)
                with (
                    tc.tile_pool(name="ln2", bufs=3) as ln2,
                    tc.tile_pool(name="ps4", bufs=2, space="PSUM") as ps4,
                ):
                    for qt in range(NQ):
                        x2_t = ln2.tile([P, D], f32, tag="x2_t")
                        nc.sync.dma_start(x2_t, x2d[qt])
                        st = ln2.tile([P, 2, 6], f32, tag="st")
                        nc.vector.bn_stats(st[:, 0, :], x2_t[:, 0:512])
                        nc.vector.bn_stats(st[:, 1, :], x2_t[:, 512:1024])
                        mv = ln2.tile([P, 2], f32, tag="mv")
                        nc.vector.bn_aggr(mv, st)
                        rstd = ln2.tile([P, 1], f32, tag="rstd")
                        nc.scalar.activation(rstd, mv[:, 1:2], AF.Sqrt, bias=eps)
                        nc.vector.reciprocal(rstd, rstd)
                        h2_t = ln2.tile([P, D], f32r, tag="h2_t")
                        nc.vector.tensor_scalar(
                            out=h2_t, in0=x2_t, scalar1=mv[:, 0:1],
                            scalar2=rstd, op0=ALU.subtract, op1=ALU.mult,
                        )
                        for dt in range(DT):
                            tp = ps4.tile([P, P], f32r, tag="tp")
                            nc.tensor.transpose(
                                tp, h2_t[:, dt * P:(dt + 1) * P], ident)
                            nc.scalar.copy(h2T[:, dt, qt * P:(qt + 1) * P], tp)

                # ---------- Phase 5: FFN ----------
                with (
                    tc.tile_pool(name="ffcst", bufs=1) as ffcp,
                    tc.tile_pool(name="ffw", bufs=3) as ffw,
                    tc.tile_pool(name="g1p", bufs=1) as g1p,
                    tc.tile_pool(name="ffo", bufs=3) as ffo,
                    tc.tile_pool(name="psa", bufs=2, space="PSUM") as psa,
                    tc.tile_pool(name="psf", bufs=1, space="PSUM") as psf,
                ):
                    b1f_sb = ffcp.tile([P, FT], f32)
                    nc.sync.dma_start(b1f_sb, b1f_d[:, :])
                    b2_bc = ffcp.tile([P, D], f32)
                    nc.sync.dma_start(b2_bc, b2_d.ap().to_broadcast([P, D]))
                    g1 = g1p.tile([P, FT, 512], f32r)
                    for qc in range(2):
                        qsl = slice(qc * 512, (qc + 1) * 512)
                        # W1 + GELU for this q chunk, all ff chunks
                        for fc in range(FT):
                            w1c = ffw.tile([P, DT, P], f32r, tag="w1c")
                            nc.sync.dma_start(
                                w1c, w1_d[:, fc * P:(fc + 1) * P].rearrange(
                                    "(dt p) m -> p dt m", p=P))
                            aps = psa.tile([P, 512], f32, tag="aps")
                            for dt in range(DT):
                                nc.tensor.matmul(
                                    aps, w1c[:, dt, :], h2T[:, dt, qsl],
                                    start=(dt == 0), stop=(dt == DT - 1))
                            nc.scalar.activation(
                                g1[:, fc, :], aps, AF.Gelu,
                                bias=b1f_sb[:, fc:fc + 1])
                        # W2 for this q chunk
                        for dh in range(2):
                            dsl = slice(dh * 512, (dh + 1) * 512)
                            fps = []
                            for j in range(4):
                                fps_j = psf.tile([P, 512], f32, tag=f"fps{j}")
                                fps.append(fps_j)
                            for fc in range(FT):
                                w2c = ffw.tile([P, 512], f32r, tag="w2c")
                                nc.sync.dma_start(
                                    w2c, w2_d[fc * P:(fc + 1) * P, dsl])
                                for j in range(4):
                                    nc.tensor.matmul(
                                        fps[j], g1[:, fc, j * P:(j + 1) * P],
                                        w2c, start=(fc == 0),
                                        stop=(fc == FT - 1))
                            for j in range(4):
                                qt = qc * 4 + j
                                o_t = ffo.tile([P, 512], f32, tag="o_t")
                                x2s = ffo.tile([P, 512], f32, tag="x2s")
                                nc.sync.dma_start(x2s, x2d[qt, :, dsl])
                                nc.vector.tensor_add(o_t, fps[j], x2s)
                                nc.vector.tensor_add(o_t, o_t, b2_bc[:, dsl])
                                nc.sync.dma_start(
                                    out_d[qt * P:(qt + 1) * P, dsl], o_t)

    nc.compile()
    return nc


def _prep_inputs(inputs):
    """Host-side: fold LN affine + score scale into weights; build per-core maps."""
    x = np.asarray(inputs["x"], dtype=np.float32)
    g1, b1_ = np.asarray(inputs["ln1_g"], np.float32), np.asarray(inputs["ln1_b"], np.float32)
    g2, b2_ = np.asarray(inputs["ln2_g"], np.float32), np.asarray(inputs["ln2_b"], np.float32)
    Wq = np.asarray(inputs["Wq"], np.float32)  # [H, D, HD]
    Wk = np.asarray(inputs["Wk"], np.float32)
    Wv = np.asarray(inputs["Wv"], np.float32)
    bq = np.asarray(inputs["bq"], np.float32)  # [H, HD]
    bk = np.asarray(inputs["bk"], np.float32)
    bv = np.asarray(inputs["bv"], np.float32)
    Wo = np.asarray(inputs["Wo"], np.float32)
    bo = np.asarray(inputs["bo"], np.float32)
    W1 = np.asarray(inputs["W1"], np.float32)
    b1 = np.asarray(inputs["b1"], np.float32)
    W2 = np.asarray(inputs["W2"], np.float32)
    b2 = np.asarray(inputs["b2"], np.float32)

    sc = 1.0 / np.sqrt(np.float32(HD))
    # [H, D, HD] -> [D, H*HD]
    wq_flat = np.transpose(Wq, (1, 0, 2)).reshape(D, D)
    wk_flat = np.transpose(Wk, (1, 0, 2)).reshape(D, D)
    wv_flat = np.transpose(Wv, (1, 0, 2)).reshape(D, D)
    wq_f = (g1[:, None] * wq_flat) * sc
    wk_f = g1[:, None] * wk_flat
    wv_f = g1[:, None] * wv_flat
    bq_f = (b1_ @ wq_flat + bq.reshape(D)) * sc
    bk_f = b1_ @ wk_flat + bk.reshape(D)
    bv_f = b1_ @ wv_flat + bv.reshape(D)
    # qkvb[hd, 3*h + {0,1,2}]
    qkvb = np.zeros((HD, 3 * H), np.float32)
    for h in range(H):
        qkvb[:, 3 * h + 0] = bq_f[h * HD:(h + 1) * HD]
        qkvb[:, 3 * h + 1] = bk_f[h * HD:(h + 1) * HD]
        qkvb[:, 3 * h + 2] = bv_f[h * HD:(h + 1) * HD]

    w1_f = g2[:, None] * W1
    b1_f = (b2_ @ W1 + b1).reshape(FF // P, P).T.copy()  # [P, FF//P]

    shared = {
        "wq": np.ascontiguousarray(wq_f), "wk": np.ascontiguousarray(wk_f),
        "wv": np.ascontiguousarray(wv_f), "wo": np.ascontiguousarray(Wo),
        "w1": np.ascontiguousarray(w1_f), "w2": np.ascontiguousarray(W2),
        "qkvb": qkvb, "bo_": bo.reshape(1, D),
        "b1f": np.ascontiguousarray(b1_f), "b2_": b2.reshape(1, D),
    }
    in_maps = []
    for c in range(8):
        b, o = c // 2, c % 2
        own = x[b, o * TQ:(o + 1) * TQ]
        oth = x[b, (1 - o) * TQ:(2 - o) * TQ]
        m = dict(shared)
        m["xl"] = np.ascontiguousarray(np.concatenate([own, oth], axis=0))
        m["h2b"] = np.array([[0.0 if o == 1 else NEG]], np.float32)
        in_maps.append(m)
    return in_maps


def kernel(**inputs):
    if "nc" not in _CACHE:
        _CACHE["nc"] = _build_program()
    nc = _CACHE["nc"]
    in_maps = _prep_inputs(inputs)
    res = run_bass_kernel_spmd(nc, in_maps, core_ids=list(range(8)))
    out = np.empty((B, T, D), np.float32)
    for c in range(8):
        b, o = c // 2, c % 2
        out[b, o * TQ:(o + 1) * TQ] = res.results[c]["out"]
    return out
